# revision 3
# baseline (speedup 1.0000x reference)
"""BinaryConv2d (sign-binarized 3x3 conv, stride 1, pad 1) on 8 Trainium2 cores.

Input  x      [32, 128, 56, 56] f32
       weight [256, 128, 3, 3]  f32  (binarized with sign() before the conv)
       b      [256]             f32
Output        [32, 256, 56, 56] f32

Sharding: data-parallel over the batch dim (4 images per core), binarized
weight replicated to all cores.

Device kernel (per core): conv implemented as 9 shift-matmuls per output
tile accumulating in PSUM. C=128 is the contraction dim (partition dim).
x is pre-padded to [128, 58, 58] so every kernel offset is a pure AP shift
and the image loads as one contiguous DMA per partition. Weights are
pre-binarized/transposed on host to lhsT layout [kh*3+kw, C, O].
Matmuls run in float32r (fp32 data, 1 cycle/row at N>=256); weights are
exactly +-1 so the only precision loss is x's internal rounding.
"""

import functools

import numpy as np

P = 128          # partitions == input channels per matmul
H = W = 56       # spatial
HP = WP = 58     # padded spatial
O = 256          # output channels
KHW = 9          # 3x3 kernel positions
HT = 8           # output rows per PSUM tile
NT = H // HT     # 7 row tiles
N_CORES = 8
N_PER_CORE = 4   # batch 32 / 8 cores


@functools.lru_cache(maxsize=1)
def _build_nc():
    import concourse.mybir as mybir
    import concourse.tile as tile
    from concourse import bacc

    nc = bacc.Bacc()
    xp = nc.declare_dram_parameter(
        "xp", [N_PER_CORE, P, HP, WP], mybir.dt.float32r, isOutput=False
    )
    wt = nc.declare_dram_parameter(
        "wt", [KHW, P, O], mybir.dt.float32r, isOutput=False
    )
    bias = nc.declare_dram_parameter("bias", [O], mybir.dt.float32, isOutput=False)
    out = nc.declare_dram_parameter(
        "out", [N_PER_CORE, O, H, W], mybir.dt.float32, isOutput=True
    )
    xp_ap = xp[:]
    wt_ap = wt[:]
    bias_ap = bias[:]
    out_ap = out[:]

    with tile.TileContext(nc) as tc:
        with (
            tc.tile_pool(name="wpool", bufs=1) as wpool,
            tc.tile_pool(name="xpool", bufs=2) as xpool,
            tc.tile_pool(name="opool", bufs=4) as opool,
            tc.tile_pool(name="psum", bufs=8, space="PSUM") as pp,
        ):
            wt_sb = wpool.tile([P, KHW, O], mybir.dt.float32r)
            nc.sync.dma_start(wt_sb[:], wt_ap.rearrange("k c o -> c k o"))
            b_sb = wpool.tile([P, 2], mybir.dt.float32)
            nc.sync.dma_start(b_sb[:], bias_ap.rearrange("(g p) -> p g", p=P))

            for n in range(N_PER_CORE):
                x_sb = xpool.tile([P, HP, WP], mybir.dt.float32r)
                nc.sync.dma_start(x_sb[:], xp_ap[n])
                for oh in range(2):
                    for t in range(NT):
                        pt = pp.tile([P, HT, W], mybir.dt.float32)
                        for kh in range(3):
                            for kw in range(3):
                                kk = kh * 3 + kw
                                nc.tensor.matmul(
                                    pt[:],
                                    wt_sb[:, kk, oh * P : (oh + 1) * P],
                                    x_sb[:, HT * t + kh : HT * t + kh + HT, kw : kw + W],
                                    start=(kk == 0),
                                    stop=(kk == KHW - 1),
                                )
                        ot = opool.tile([P, HT, W], mybir.dt.float32)
                        nc.scalar.add(ot[:], pt[:], b_sb[:, oh : oh + 1])
                        nc.sync.dma_start(
                            out_ap[n, oh * P : (oh + 1) * P, HT * t : HT * t + HT, :],
                            ot[:],
                        )
    nc.finalize()
    return nc


def _prep(x, weight, b):
    x = np.asarray(x, dtype=np.float32)
    w = np.asarray(weight, dtype=np.float32)
    b = np.ascontiguousarray(np.asarray(b, dtype=np.float32))
    bw = np.sign(w)  # matches torch/jax sign: sign(0) = 0
    # [O, C, kh, kw] -> [kh*3+kw, C, O] (lhsT layout: contraction on partitions)
    wt = np.ascontiguousarray(bw.transpose(2, 3, 1, 0).reshape(KHW, P, O))
    xp = np.zeros((x.shape[0], P, HP, WP), np.float32)
    xp[:, :, 1 : H + 1, 1 : W + 1] = x
    return xp, wt, b


def _run(in_maps, trace=False):
    from concourse.bass_utils import run_bass_kernel_spmd

    nc = _build_nc()
    return run_bass_kernel_spmd(
        nc, in_maps, core_ids=list(range(N_CORES)), trace=trace
    )


def kernel(x, weight, b):
    xp, wt, bias = _prep(x, weight, b)
    in_maps = [
        {
            "xp": np.ascontiguousarray(xp[c * N_PER_CORE : (c + 1) * N_PER_CORE]),
            "wt": wt,
            "bias": bias,
        }
        for c in range(N_CORES)
    ]
    res = _run(in_maps, trace=False)
    return np.concatenate([r["out"] for r in res.results], axis=0)


# revision 5
# speedup vs baseline: 1.0152x; 1.0152x over previous
"""BinaryConv2d (sign-binarized 3x3 conv, stride 1, pad 1) on 8 Trainium2 cores.

Input  x      [32, 128, 56, 56] f32
       weight [256, 128, 3, 3]  f32  (binarized with sign() before the conv)
       b      [256]             f32
Output        [32, 256, 56, 56] f32

Sharding: data-parallel over the batch dim (4 images per core), binarized
weight replicated to all cores.

Device kernel (per core): conv implemented as 9 shift-matmuls per output
tile accumulating in PSUM. C=128 is the contraction dim (partition dim).
x is pre-padded to [128, 58, 58] so every kernel offset is a pure AP shift
and the image loads as one contiguous DMA per partition. Weights are
pre-binarized/transposed on host to lhsT layout [kh*3+kw, C, O].
Matmuls run in float32r (fp32 data, 1 cycle/row at N>=256); weights are
exactly +-1 so the only precision loss is x's internal rounding.
"""

import functools

import numpy as np

P = 128          # partitions == input channels per matmul
H = W = 56       # spatial
HP = WP = 58     # padded spatial
O = 256          # output channels
KHW = 9          # 3x3 kernel positions
HT = 8           # output rows per PSUM tile
NT = H // HT     # 7 row tiles
N_CORES = 8
N_PER_CORE = 4   # batch 32 / 8 cores


@functools.lru_cache(maxsize=1)
def _build_nc():
    import concourse.mybir as mybir
    import concourse.tile as tile
    from concourse import bacc

    nc = bacc.Bacc()
    xp = nc.declare_dram_parameter(
        "xp", [N_PER_CORE, P, HP, WP], mybir.dt.float32r, isOutput=False
    )
    wt = nc.declare_dram_parameter(
        "wt", [KHW, P, O], mybir.dt.float32r, isOutput=False
    )
    bias = nc.declare_dram_parameter("bias", [O], mybir.dt.float32, isOutput=False)
    out = nc.declare_dram_parameter(
        "out", [N_PER_CORE, O, H, W], mybir.dt.float32, isOutput=True
    )
    xp_ap = xp[:]
    wt_ap = wt[:]
    bias_ap = bias[:]
    out_ap = out[:]

    with tile.TileContext(nc) as tc:
        with (
            tc.tile_pool(name="wpool", bufs=1) as wpool,
            tc.tile_pool(name="xpool", bufs=2 * NT) as xpool,
            tc.tile_pool(name="opool", bufs=4) as opool,
            tc.tile_pool(name="psum", bufs=8, space="PSUM") as pp,
        ):
            wt_sb = wpool.tile([P, KHW, O], mybir.dt.float32r)
            nc.sync.dma_start(wt_sb[:], wt_ap.rearrange("k c o -> c k o"))
            b_sb = wpool.tile([P, 2], mybir.dt.float32)
            nc.sync.dma_start(b_sb[:], bias_ap.rearrange("(g p) -> p g", p=P))

            for n in range(N_PER_CORE):
                # Load the padded image as NT halo chunks (rows 8t .. 8t+9)
                # so the first matmul group only waits on its own 10 rows,
                # not the whole 1.7 MB image.
                chunks = []
                for t in range(NT):
                    xc = xpool.tile([P, HT + 2, WP], mybir.dt.float32r, tag="xc")
                    nc.sync.dma_start(xc[:], xp_ap[n, :, HT * t : HT * t + HT + 2, :])
                    chunks.append(xc)
                for oh in range(2):
                    for t in range(NT):
                        x_sb = chunks[t]
                        pt = pp.tile([P, HT, W], mybir.dt.float32)
                        for kh in range(3):
                            for kw in range(3):
                                kk = kh * 3 + kw
                                nc.tensor.matmul(
                                    pt[:],
                                    wt_sb[:, kk, oh * P : (oh + 1) * P],
                                    x_sb[:, kh : kh + HT, kw : kw + W],
                                    start=(kk == 0),
                                    stop=(kk == KHW - 1),
                                )
                        ot = opool.tile([P, HT, W], mybir.dt.float32)
                        nc.scalar.add(ot[:], pt[:], b_sb[:, oh : oh + 1])
                        nc.sync.dma_start(
                            out_ap[n, oh * P : (oh + 1) * P, HT * t : HT * t + HT, :],
                            ot[:],
                        )
    nc.finalize()
    return nc


def _prep(x, weight, b):
    x = np.asarray(x, dtype=np.float32)
    w = np.asarray(weight, dtype=np.float32)
    b = np.ascontiguousarray(np.asarray(b, dtype=np.float32))
    bw = np.sign(w)  # matches torch/jax sign: sign(0) = 0
    # [O, C, kh, kw] -> [kh*3+kw, C, O] (lhsT layout: contraction on partitions)
    wt = np.ascontiguousarray(bw.transpose(2, 3, 1, 0).reshape(KHW, P, O))
    xp = np.zeros((x.shape[0], P, HP, WP), np.float32)
    xp[:, :, 1 : H + 1, 1 : W + 1] = x
    return xp, wt, b


def _run(in_maps, trace=False):
    from concourse.bass_utils import run_bass_kernel_spmd

    nc = _build_nc()
    return run_bass_kernel_spmd(
        nc, in_maps, core_ids=list(range(N_CORES)), trace=trace
    )


def kernel(x, weight, b):
    xp, wt, bias = _prep(x, weight, b)
    in_maps = [
        {
            "xp": np.ascontiguousarray(xp[c * N_PER_CORE : (c + 1) * N_PER_CORE]),
            "wt": wt,
            "bias": bias,
        }
        for c in range(N_CORES)
    ]
    res = _run(in_maps, trace=False)
    return np.concatenate([r["out"] for r in res.results], axis=0)
